# revision 7
# baseline (speedup 1.0000x reference)
"""Trainium2 Bass kernel for AdaptiveGraphNetwork (2x SAGEConv + GAT + edge MLP).

Self-contained: host-side edge partitioning/sorting + an 8-core SPMD Bass
kernel (dma_gather + one-hot-matmul segment aggregation + AllGather).
"""
import numpy as np
import ml_dtypes

import concourse.bacc as bacc
import concourse.bass as bass
import concourse.mybir as mybir
import concourse.tile as tile
from concourse.bass_utils import run_bass_kernel_spmd
from concourse.masks import make_identity
from concourse._compat import cdiv

P = 8                  # cores
N = 100000             # nodes
NPART = 12500          # real nodes per core
NP = 12544             # padded nodes per core (98 windows x 128)
NW = 98                # dst windows per core
NTAB = NP * P          # 100352 gatherable table rows
CHUNK = 25088          # table rows per gather chunk (int16 range)
NCHUNK = 4
MAXCALL = 1024         # max idxs per dma_gather
F32 = mybir.dt.float32
BF16 = mybir.dt.bfloat16
I16 = mybir.dt.int16
AF = mybir.ActivationFunctionType
ALU = mybir.AluOpType

LAST_EXEC_NS = None


def _roundup(a, m):
    return (a + m - 1) // m * m


def _pack_idx_block(idx):
    """1024-or-less idx array (len%16==0) -> [128, len/16] int16 wrapped+replicated."""
    n = len(idx)
    w = idx.astype(np.int16).reshape(n // 16, 16).T  # [16, n/16]
    return np.tile(w, (8, 1))


def _host_prep(edge_index):
    src = np.asarray(edge_index[0]).astype(np.int64)
    dst = np.asarray(edge_index[1]).astype(np.int64)
    E = src.shape[0]
    c_of = dst // NPART
    dl = dst % NPART
    wi = dl // 128          # window within core
    dw = dl % 128           # id within window
    srow = (src // NPART) * NP + (src % NPART)   # gather-table row
    kc = srow // CHUNK      # chunk
    ridx = srow % CHUNK     # idx within chunk

    # counts[c, w, k]
    counts = np.zeros((P, NW, NCHUNK), np.int64)
    eid_by = {}
    for c in range(P):
        m = np.nonzero(c_of == c)[0]
        order = np.lexsort((kc[m], wi[m]))
        ms = m[order]
        eid_by[c] = ms
        np.add.at(counts[c], (wi[ms], kc[ms]), 1)
    C = _roundup(counts.max(axis=0), 128)  # [NW, NCHUNK] uniform capacities
    S = int(C.sum())                        # total slots per core
    NT = S // 128                           # tiles per core

    # slot arrays per core
    idx16 = np.zeros((P, S), np.int16)
    dstw = np.full((P, S), -1.0, np.float32)
    perm = np.full((P, S), -1, np.int64)
    # fragment offsets (same all cores)
    frag_off = np.zeros((NW, NCHUNK), np.int64)
    off = 0
    for w in range(NW):
        for k in range(NCHUNK):
            frag_off[w, k] = off
            off += C[w, k]
    for c in range(P):
        ms = eid_by[c]
        wv, kv = wi[ms], kc[ms]
        # position within fragment
        pos = np.zeros(len(ms), np.int64)
        cnt = np.zeros((NW, NCHUNK), np.int64)
        for i in range(len(ms)):
            a, b = wv[i], kv[i]
            pos[i] = cnt[a, b]
            cnt[a, b] += 1
        slot = frag_off[wv, kv] + pos
        idx16[c, slot] = ridx[ms].astype(np.int16)
        dstw[c, slot] = dw[ms].astype(np.float32)
        perm[c, slot] = ms

    # gather calls: per (w, k) fragment split at MAXCALL
    calls = []  # (chunk, n_idxs, slot_off, col_off)
    col = 0
    for w in range(NW):
        for k in range(NCHUNK):
            rem, so = int(C[w, k]), int(frag_off[w, k])
            while rem > 0:
                n = min(rem, MAXCALL)
                calls.append((k, n, so, col))
                col += n // 16
                so += n
                rem -= n
    ncols = col

    # packed idx input [P, 128, ncols]
    idx_in = np.zeros((P, 128, ncols), np.int16)
    for c in range(P):
        for (k, n, so, co) in calls:
            idx_in[c, :, co:co + n // 16] = _pack_idx_block(idx16[c, so:so + n])
    # dstw input [P, 128, NT]: tile t col t, partition p = slot t*128+p
    dstw_in = dstw.reshape(P, NT, 128).transpose(0, 2, 1).copy()

    # window of each tile + first/last tile per window
    tile_w = np.zeros(NT, np.int64)
    for w in range(NW):
        a = frag_off[w, 0] // 128
        b = (frag_off[w, 3] + C[w, 3]) // 128
        tile_w[a:b] = w
    w_first = {}
    w_last = {}
    for t in range(NT):
        w = int(tile_w[t])
        if w not in w_first:
            w_first[w] = t
        w_last[w] = t
    return dict(calls=calls, idx_in=idx_in, dstw_in=dstw_in, S=S, NT=NT,
                tile_w=tile_w, w_first=w_first, w_last=w_last, perm=perm,
                ncols=ncols)


def _build_graph(prep, scalars):
    """Build the SPMD Bass graph. scalars: dict with att_src, att_dst, bg, be2."""
    calls, NT = prep["calls"], prep["NT"]
    ncols = prep["ncols"]
    tile_w, w_first, w_last = prep["tile_w"], prep["w_first"], prep["w_last"]
    att_src, att_dst, bg_s, be2_s = (float(scalars[k]) for k in
                                     ("att_src", "att_dst", "bg", "be2"))

    nc = bacc.Bacc("TRN2", num_swdge_queues=4)
    dp = nc.declare_dram_parameter
    xpad = dp("xpad", [NTAB, 128], BF16, isOutput=False)       # x table (col7=1)
    xT = dp("xT", [7, NP], BF16, isOutput=False)               # own x transposed
    idx_d = dp("idx", [128, ncols], I16, isOutput=False)
    dstw_d = dp("dstw", [128, NT], F32, isOutput=False)
    wts = {}
    for name, shape in (("w1l", [7, 128]), ("w1r", [7, 128]), ("b1r", [1, 128]),
                        ("w2l", [128, 128]), ("w2r", [128, 128]), ("b2r", [1, 128]),
                        ("wv1", [128, 128]), ("bv1r", [1, 128]),
                        ("wv2", [128, 3]), ("bv2r", [1, 3]),
                        ("we1t", [128, 128]), ("we1b", [128, 128]), ("be1r", [1, 128]),
                        ("wg", [128, 1]), ("we2bc", [128, 128])):
        wts[name] = dp(name, shape, BF16, isOutput=False)
    pos_o = dp("pos", [NP, 3], F32, isOutput=True)
    ew_o = dp("ew", [NP, 1], F32, isOutput=True)
    conn_o = dp("conn", [128, NT], F32, isOutput=True)

    h1_shard = nc.dram_tensor("h1_shard", [NP, 128], BF16)
    h1_full = nc.dram_tensor("h1_full", [NTAB, 128], BF16)
    t2_shard = nc.dram_tensor("t2_shard", [NP, 256], BF16)
    t2_full = nc.dram_tensor("t2_full", [NTAB, 256], BF16)

    with tile.TileContext(nc) as tc:
        cpool = tc.alloc_tile_pool(name="const", bufs=1)
        spool = tc.alloc_tile_pool(name="stage", bufs=8)
        wpool = tc.alloc_tile_pool(name="work", bufs=3)
        bpool = tc.alloc_tile_pool(name="big", bufs=1)
        p1pool = tc.alloc_tile_pool(name="psum1", bufs=2, space="PSUM")
        p2pool = tc.alloc_tile_pool(name="psum2", bufs=2, space="PSUM")
        opool = tc.alloc_tile_pool(name="outp", bufs=4)

        # ---- constants / persistent ----
        idx_sb = bpool.tile([128, ncols], I16, tag="idx", name="idx")
        nc.sync.dma_start(idx_sb[:], idx_d[:])
        dstw_sb = bpool.tile([128, NT], F32, tag="dstw", name="dstw")
        nc.sync.dma_start(dstw_sb[:], dstw_d[:])
        xT_sb = cpool.tile([7, NP], BF16, tag="xT", name="xT")
        nc.sync.dma_start(xT_sb[:], xT[:])
        wt_sb = {}
        for name in wts:
            t = cpool.tile(list(wts[name].shape), BF16, tag=name)
            nc.sync.dma_start(t[:], wts[name][:])
            wt_sb[name] = t
        iota_row = cpool.tile([128, 128], F32, tag="iota", name="iota")
        nc.gpsimd.iota(iota_row[:], pattern=[[1, 128]], base=0, channel_multiplier=0,
                       allow_small_or_imprecise_dtypes=True)
        ident = cpool.tile([128, 128], BF16, tag="ident", name="ident")
        make_identity(nc, ident[:])
        ones_r = cpool.tile([1, 128], BF16, tag="ones", name="ones")
        nc.vector.memset(ones_r[:], 1.0)
        degrecip = bpool.tile([128, NW], F32, tag="degrecip", name="degrecip")
        h1_sb = bpool.tile([128, NW * 128], BF16, tag="h1sb", name="h1sb")
        b2ad_sb = bpool.tile([128, NW * 129], BF16, tag="b2ad", name="b2ad")
        conn_sb = bpool.tile([128, NT], F32, tag="connsb", name="connsb")

        chunk_x = [xpad[CHUNK * k:CHUNK * (k + 1), :] for k in range(NCHUNK)]
        chunk_h1 = [h1_full[CHUNK * k:CHUNK * (k + 1), :] for k in range(NCHUNK)]
        chunk_t2 = [t2_full[CHUNK * k:CHUNK * (k + 1), :] for k in range(NCHUNK)]

        def build_S(st_S, t):
            nc.vector.tensor_scalar(st_S[:], iota_row[:], dstw_sb[:, t:t + 1], None,
                                    ALU.is_equal)

        def gather_phase(tag, chunks, elem, per_tile, per_window_end, qbase=0):
            state = {}
            for ci, (k, n, so, co) in enumerate(calls):
                st = spool.tile([128, n // 128, elem], BF16, tag=tag)
                nc.gpsimd.dma_gather(st[:], chunks[k], idx_sb[:, co:co + n // 16],
                                     n, n, elem, queue_num=(qbase + ci) % 4)
                for b in range(n // 128):
                    t = so // 128 + b
                    w = int(tile_w[t])
                    first, last = w_first[w] == t, w_last[w] == t
                    per_tile(t, w, st[:, b, :], first, last, state)
                    if last:
                        per_window_end(w, state)

        # ================= Phase A: conv1 =================
        def a_tile(t, w, M, first, last, state):
            S = wpool.tile([128, 128], BF16, tag="S", name="S")
            build_S(S, t)
            if first:
                state["ps"] = p1pool.tile([128, 8], F32, tag="agg", name="agg")
            nc.tensor.matmul(state["ps"][:], lhsT=S[:], rhs=M[:, 0:8],
                             start=first, stop=last)

        def a_wend(w, state):
            ps = state["ps"]
            deg = wpool.tile([128, 1], F32, tag="deg", name="deg")
            nc.vector.tensor_scalar(deg[:], ps[:, 7:8], 1.0, None, ALU.max)
            nc.vector.reciprocal(degrecip[:, w:w + 1], deg[:])
            m1 = wpool.tile([128, 8], BF16, tag="m1", name="m1")
            nc.vector.tensor_scalar(m1[:, 0:7], ps[:, 0:7], degrecip[:, w:w + 1],
                                    None, ALU.mult)
            m1t_p = p2pool.tile([128, 128], BF16, tag="tr", name="tr")
            nc.tensor.transpose(m1t_p[:7, :], m1[:, 0:7], ident[:])
            m1t = wpool.tile([7, 128], BF16, tag="m1t", name="m1t")
            nc.scalar.copy(m1t[:], m1t_p[:7, :])
            hp = p2pool.tile([128, 128], F32, tag="dense", name="dense")
            nc.tensor.matmul(hp[:], lhsT=m1t[:], rhs=wt_sb["w1l"][:], start=True, stop=False)
            nc.tensor.matmul(hp[:], lhsT=xT_sb[:, w * 128:(w + 1) * 128],
                             rhs=wt_sb["w1r"][:], start=False, stop=False)
            nc.tensor.matmul(hp[:], lhsT=ones_r[:], rhs=wt_sb["b1r"][:], start=False, stop=True)
            hw = h1_sb[:, w * 128:(w + 1) * 128]
            nc.scalar.activation(hw, hp[:], AF.Relu)
            nc.sync.dma_start(h1_shard[w * 128:(w + 1) * 128, :], hw)

        gather_phase("stA", chunk_x, 128, a_tile, a_wend)

        # AllGather h1
        nc.gpsimd.collective_compute(
            "AllGather", ALU.bypass, ins=[h1_shard.ap().opt()],
            outs=[h1_full.ap().opt()], replica_groups=[list(range(P))])

        # ================= Phase B: conv2 + node-dense =================
        def b_tile(t, w, M, first, last, state):
            S = wpool.tile([128, 128], BF16, tag="S", name="S")
            build_S(S, t)
            if first:
                state["ps"] = p1pool.tile([128, 128], F32, tag="agg", name="agg")
            nc.tensor.matmul(state["ps"][:], lhsT=S[:], rhs=M[:], start=first, stop=last)

        def b_wend(w, state):
            ps = state["ps"]
            wsl = slice(w * 128, (w + 1) * 128)
            m2 = wpool.tile([128, 128], BF16, tag="m2", name="m2")
            nc.vector.tensor_scalar(m2[:], ps[:], degrecip[:, w:w + 1], None, ALU.mult)
            m2t_p = p2pool.tile([128, 128], BF16, tag="tr", name="tr")
            nc.tensor.transpose(m2t_p[:], m2[:], ident[:])
            m2t = wpool.tile([128, 128], BF16, tag="m2t", name="m2t")
            nc.scalar.copy(m2t[:], m2t_p[:])
            h1t_p = p2pool.tile([128, 128], BF16, tag="tr", name="tr")
            nc.tensor.transpose(h1t_p[:], h1_sb[:, wsl], ident[:])
            h1t = wpool.tile([128, 128], BF16, tag="h1t", name="h1t")
            nc.scalar.copy(h1t[:], h1t_p[:])
            hp = p2pool.tile([128, 128], F32, tag="dense", name="dense")
            nc.tensor.matmul(hp[:], lhsT=m2t[:], rhs=wt_sb["w2l"][:], start=True, stop=False)
            nc.tensor.matmul(hp[:], lhsT=h1t[:], rhs=wt_sb["w2r"][:], start=False, stop=False)
            nc.tensor.matmul(hp[:], lhsT=ones_r[:], rhs=wt_sb["b2r"][:], start=False, stop=True)
            h2 = wpool.tile([128, 128], BF16, tag="h2", name="h2")
            nc.scalar.copy(h2[:], hp[:])
            h2t_p = p2pool.tile([128, 128], BF16, tag="tr", name="tr")
            nc.tensor.transpose(h2t_p[:], h2[:], ident[:])
            h2t = wpool.tile([128, 128], BF16, tag="h2t", name="h2t")
            nc.scalar.copy(h2t[:], h2t_p[:])
            # positions
            t1p = p2pool.tile([128, 128], F32, tag="dense", name="dense")
            nc.tensor.matmul(t1p[:], lhsT=h2t[:], rhs=wt_sb["wv1"][:], start=True, stop=False)
            nc.tensor.matmul(t1p[:], lhsT=ones_r[:], rhs=wt_sb["bv1r"][:], start=False, stop=True)
            t1 = wpool.tile([128, 128], BF16, tag="t1", name="t1")
            nc.scalar.activation(t1[:], t1p[:], AF.Relu)
            t1t_p = p2pool.tile([128, 128], BF16, tag="tr", name="tr")
            nc.tensor.transpose(t1t_p[:], t1[:], ident[:])
            t1t = wpool.tile([128, 128], BF16, tag="t1t", name="t1t")
            nc.scalar.copy(t1t[:], t1t_p[:])
            pp = p2pool.tile([128, 3], F32, tag="dense", name="dense")
            nc.tensor.matmul(pp[:], lhsT=t1t[:], rhs=wt_sb["wv2"][:], start=True, stop=False)
            nc.tensor.matmul(pp[:], lhsT=ones_r[:], rhs=wt_sb["bv2r"][:], start=False, stop=True)
            pos_t = opool.tile([128, 3], F32, tag="post", name="post")
            nc.vector.tensor_copy(pos_t[:], pp[:])
            nc.sync.dma_start(pos_o[w * 128:(w + 1) * 128, :], pos_t[:])
            # A2 | q | a_s  -> t2 table
            a2p = p2pool.tile([128, 128], F32, tag="dense", name="dense")
            nc.tensor.matmul(a2p[:], lhsT=h2t[:], rhs=wt_sb["we1t"][:], start=True, stop=False)
            nc.tensor.matmul(a2p[:], lhsT=ones_r[:], rhs=wt_sb["be1r"][:], start=False, stop=True)
            qp = p2pool.tile([128, 1], F32, tag="dense", name="dense")
            nc.tensor.matmul(qp[:], lhsT=h2t[:], rhs=wt_sb["wg"][:], start=True, stop=True)
            t2t = opool.tile([128, 256], BF16, tag="t2t", name="t2t")
            nc.scalar.copy(t2t[:, 0:128], a2p[:])
            nc.vector.tensor_copy(t2t[:, 128:130].bitcast(F32), qp[:])
            nc.vector.tensor_scalar(t2t[:, 130:132].bitcast(F32), qp[:], att_src,
                                    None, ALU.mult)
            nc.vector.memset(t2t[:, 132:256], 0.0)
            nc.sync.dma_start(t2_shard[w * 128:(w + 1) * 128, :], t2t[:])
            # B2 | a_d
            b2p = p2pool.tile([128, 128], F32, tag="dense", name="dense")
            nc.tensor.matmul(b2p[:], lhsT=h2t[:], rhs=wt_sb["we1b"][:], start=True, stop=True)
            bsl = slice(w * 129, w * 129 + 128)
            nc.scalar.copy(b2ad_sb[:, bsl], b2p[:])
            nc.vector.tensor_scalar(b2ad_sb[:, w * 129 + 128:w * 129 + 129], qp[:],
                                    att_dst, None, ALU.mult)

        gather_phase("stB", chunk_h1, 128, b_tile, b_wend)

        # AllGather t2
        nc.gpsimd.collective_compute(
            "AllGather", ALU.bypass, ins=[t2_shard.ap().opt()],
            outs=[t2_full.ap().opt()], replica_groups=[list(range(P))])

        # ================= Phase C: edge MLP + GAT =================
        def c_tile(t, w, M, first, last, state):
            S = wpool.tile([128, 128], BF16, tag="S", name="S")
            build_S(S, t)
            st_p = p2pool.tile([128, 128], BF16, tag="tr", name="tr")
            nc.tensor.transpose(st_p[:], S[:], ident[:])
            S_T = wpool.tile([128, 128], BF16, tag="ST", name="ST")
            nc.scalar.copy(S_T[:], st_p[:])
            b2e = p2pool.tile([128, 129], F32, tag="dense", name="dense")
            nc.tensor.matmul(b2e[:], lhsT=S_T[:], rhs=b2ad_sb[:, w * 129:(w + 1) * 129],
                             start=True, stop=True)
            mlp1 = wpool.tile([128, 128], BF16, tag="mlp1", name="mlp1")
            nc.vector.tensor_add(mlp1[:], M[:, 0:128], b2e[:, 0:128])
            mlp2 = wpool.tile([128, 128], BF16, tag="mlp2", name="mlp2")
            nc.scalar.activation(mlp2[:], mlp1[:], AF.Relu)
            mlp3 = wpool.tile([128, 128], BF16, tag="mlp3", name="mlp3")
            nc.vector.tensor_tensor(mlp3[:], mlp2[:], wt_sb["we2bc"][:], ALU.mult)
            red = wpool.tile([128, 1], F32, tag="red", name="red")
            nc.vector.tensor_reduce(red[:], mlp3[:], mybir.AxisListType.X, ALU.add)
            nc.scalar.activation(conn_sb[:, t:t + 1], red[:], AF.Sigmoid, bias=be2_s)
            # GAT
            asum = wpool.tile([128, 1], F32, tag="asum", name="asum")
            nc.vector.tensor_add(asum[:], M[:, 130:132].bitcast(F32), b2e[:, 128:129])
            gat2 = wpool.tile([128, 2], BF16, tag="gat2", name="gat2")
            lr = wpool.tile([128, 1], F32, tag="lr", name="lr")
            nc.scalar.activation(lr[:], asum[:], AF.Lrelu, alpha=0.2)
            nc.scalar.activation(gat2[:, 0:1], lr[:], AF.Exp)
            nc.vector.tensor_tensor(gat2[:, 1:2], gat2[:, 0:1], M[:, 128:130].bitcast(F32), ALU.mult)
            if first:
                state["pg"] = p1pool.tile([128, 2], F32, tag="agg", name="agg")
            nc.tensor.matmul(state["pg"][:], lhsT=S[:], rhs=gat2[:], start=first, stop=last)

        def c_wend(w, state):
            pg = state["pg"]
            den = wpool.tile([128, 1], F32, tag="den", name="den")
            nc.vector.tensor_scalar(den[:], pg[:, 0:1], 1e-16, None, ALU.add)
            deni = wpool.tile([128, 1], F32, tag="deni", name="deni")
            nc.vector.reciprocal(deni[:], den[:])
            ew_t = opool.tile([128, 1], F32, tag="ewt", name="ewt")
            nc.vector.tensor_tensor(ew_t[:], pg[:, 1:2], deni[:], ALU.mult)
            nc.vector.tensor_scalar(ew_t[:], ew_t[:], bg_s, None, ALU.add)
            nc.sync.dma_start(ew_o[w * 128:(w + 1) * 128, :], ew_t[:])

        gather_phase("stC", chunk_t2, 256, c_tile, c_wend)
        nc.sync.dma_start(conn_o[:], conn_sb[:])

        for p in (opool, p2pool, p1pool, bpool, wpool, spool, cpool):
            p.release()
    nc.compile()
    return nc


def kernel(x, edge_index, edge_attr,
           w1_l, b1, w1_r, w2_l, b2, w2_r,
           wg, att_src, att_dst, bg,
           wv1, bv1, wv2, bv2, we1, be1, we2, be2, _trace=False):
    global LAST_EXEC_NS
    x = np.asarray(x, np.float32)
    E = np.asarray(edge_index).shape[1]
    prep = _host_prep(np.asarray(edge_index))

    bf = ml_dtypes.bfloat16
    # x table [NTAB, 128] bf16 (cols 0:7 x, col 7 = 1.0)
    xpad = np.zeros((NTAB, 128), bf)
    xs = np.zeros((NTAB, 7), np.float32)
    ones = np.zeros((NTAB,), np.float32)
    for c in range(P):
        xs[c * NP:c * NP + NPART] = x[c * NPART:(c + 1) * NPART]
        ones[c * NP:c * NP + NPART] = 1.0
    xpad[:, 0:7] = xs.astype(bf)
    xpad[:, 7] = ones.astype(bf)

    scalars = dict(att_src=np.float32(att_src[0]), att_dst=np.float32(att_dst[0]),
                   bg=np.float32(bg[0]), be2=np.float32(be2[0]))
    nc = _build_graph(prep, scalars)

    we1 = np.asarray(we1, np.float32)
    wt_np = {
        "w1l": np.asarray(w1_l), "w1r": np.asarray(w1_r),
        "b1r": np.asarray(b1)[None, :],
        "w2l": np.asarray(w2_l), "w2r": np.asarray(w2_r),
        "b2r": np.asarray(b2)[None, :],
        "wv1": np.asarray(wv1), "bv1r": np.asarray(bv1)[None, :],
        "wv2": np.asarray(wv2), "bv2r": np.asarray(bv2)[None, :],
        "we1t": we1[:128], "we1b": we1[128:],
        "be1r": np.asarray(be1)[None, :],
        "wg": np.asarray(wg),
        "we2bc": np.tile(np.asarray(we2)[:, 0][None, :], (128, 1)),
    }
    in_maps = []
    for c in range(P):
        xTc = np.zeros((7, NP), np.float32)
        xTc[:, :NPART] = x[c * NPART:(c + 1) * NPART].T
        im = {"xpad": xpad, "xT": xTc.astype(bf),
              "idx": prep["idx_in"][c], "dstw": prep["dstw_in"][c],
              "pos": np.zeros((NP, 3), np.float32),
              "ew": np.zeros((NP, 1), np.float32),
              "conn": np.zeros((128, prep["NT"]), np.float32)}
        for k, v in wt_np.items():
            im[k] = v.astype(bf)
        in_maps.append(im)

    res = run_bass_kernel_spmd(nc, in_maps, core_ids=list(range(P)), trace=_trace)
    LAST_EXEC_NS = res.exec_time_ns

    pos = np.zeros((N, 3), np.float32)
    ew = np.zeros((N, 1), np.float32)
    conn = np.zeros((E, 1), np.float32)
    for c in range(P):
        r = res.results[c]
        pos[c * NPART:(c + 1) * NPART] = r["pos"][:NPART]
        ew[c * NPART:(c + 1) * NPART] = r["ew"][:NPART]
        flat = r["conn"].T.reshape(-1)       # slot-ordered
        pm = prep["perm"][c]
        m = pm >= 0
        conn[pm[m], 0] = flat[m]
    return (pos, conn, ew)
